# revision 1
# baseline (speedup 1.0000x reference)
"""Multi-head causal self-attention on 8 Trainium2 NeuronCores.

Problem: B=4, S=2048, D=1024, H=16 heads (dk=64), fp32 in/out, causal softmax.

Sharding: hybrid batch x head-group. Core c handles batch b = c//2 and head
group g = c%2 (8 heads = 512 dims). Each core computes QKV projections for
its head group, causal attention, and a partial output projection over its
512 context dims. The host sums the two bf16 partials per batch in fp32.

Device-side design (per core), all matmul operands bf16 (PSUM fp32):
  - Cost model charges matmuls by output free-size only, so every matmul is
    arranged to produce 128 output partitions per moving row where possible.
  - Q^T/K^T [128, pair, S]: partition block = head pair (64 rows each head).
  - Scores S^T[k, q] per (q-tile 128, k-chunk 128): 8 matmuls (contraction
    dk=64) into one 2-bank PSUM tile [128, 8head, 128q]; one exp (ACT) over
    all 8 heads; triangular mask multiply on the diagonal chunk only.
  - P@V runs TRANSPOSED: out ctx[q 128, 8, dv 64+1] with lhsT = exp(S^T)
    [k,q] and rhs = V_aug [k, 65] (ones column -> softmax denominator), so
    each PV matmul moves only 65 rows instead of 512.  The ctx accumulator
    PSUM tile is memset-zeroed once per q-tile and all PV matmuls use
    start=False (multiple interleaved accumulation groups share banks; the
    2KB zeroing granularity of start=True would clobber neighbours).
  - Normalization is a per-partition broadcast multiply on DVE (q is the
    partition dim), then one PE transpose per head PAIR ([q,2*64] -> pair
    layout [128, q]) readies ctx^T as the lhsT of the output projection.
  - Output projection accumulates 4 pairs x 512 cols in PSUM; bf16 partial
    written to DRAM; host adds the two head-group partials per batch.
  - Schedule: forward q-tile sweep with K/Q/V projection groups and
    deferred output projections interleaved into the attention loops as PE
    filler under the exp-bound (ACT) stretches.
"""

import numpy as np
from contextlib import ExitStack

import concourse.bass as bass
import concourse.tile as tile
from concourse import bacc, mybir
from concourse.bass_utils import run_bass_kernel_spmd

B, S, D = 4, 2048, 1024
H16 = 16
DK = 64
G = 2               # head groups (cores per batch)
HD = D // G         # per-core head dims = 512 (8 heads)
NH = HD // DK       # heads per core = 8
NJ = NH // 2        # head pairs per core = 4
P = 128
NQS = S // P        # 16 q subtiles
NKC = S // P        # 16 k chunks
KO = D // P         # 8 contraction chunks for projections
QC = 512            # projection s-chunk

F32 = mybir.dt.float32
BF16 = mybir.dt.bfloat16
EXP = mybir.ActivationFunctionType.Exp

_CACHE: dict = {}


def _emit(ctx: ExitStack, tc, xt, wq, wk, wv, wo, tri, ident, out):
    nc = tc.nc

    persist = ctx.enter_context(tc.tile_pool(name="persist", bufs=1))
    xt_sb = persist.tile([P, KO, S], BF16)
    wq_sb = persist.tile([P, KO, HD], BF16)
    wk_sb = persist.tile([P, KO, HD], BF16)
    wv_sb = persist.tile([P, KO, HD], BF16)
    wo_sb = persist.tile([P, NJ, D], BF16)
    qt_ev = persist.tile([P, NJ, S], BF16)
    qt_od = persist.tile([P, NJ, S], BF16)
    kt_sb = persist.tile([P, NJ, S], BF16)
    v_sb = persist.tile([P, NKC, NH, DK + 1], BF16)
    ctxT_sb = persist.tile([P, NJ, NQS, P], BF16)
    tri_sb = persist.tile([P, 1, P], BF16)
    id_sb = persist.tile([P, P], BF16)
    warm_sb = persist.tile([P, 2], BF16)

    xt_r = xt.rearrange("(o p) s -> p o s", p=P)
    wq_r = wq.rearrange("(o p) m -> p o m", p=P)
    wk_r = wk.rearrange("(o p) m -> p o m", p=P)
    wv_r = wv.rearrange("(o p) m -> p o m", p=P)
    wo_r = wo.rearrange("(j p) o -> p j o", p=P)

    with (
        tc.tile_pool(name="spps", bufs=2, space="PSUM") as spps,
        tc.tile_pool(name="cxps", bufs=1, space="PSUM") as cxps,
        tc.tile_pool(name="mixps", bufs=2, space="PSUM") as mixps,
        tc.tile_pool(name="ptp", bufs=3) as ptp,
        tc.tile_pool(name="cnp", bufs=2) as cnp,
        tc.tile_pool(name="rrp", bufs=2) as rrp,
        tc.tile_pool(name="osb", bufs=3) as osb,
    ):
        # ---- input DMAs, interleaved so the first K-proj group is fully fed
        # by ~2.5us.  The prologue needs only wk + the first s-chunk of x, so
        # x's sc0 columns load first, the remainder streams in behind.
        # queues: SP wk/consts/wq/wo; Pool + ACT split x chunks; Pool wv.
        for ko in range(4):
            nc.sync.dma_start(wk_sb[:, ko, :], wk_r[:, ko, :])
        for ko in range(0, KO, 2):
            nc.gpsimd.dma_start(xt_sb[:, ko, 0:QC], xt_r[:, ko, 0:QC])
            nc.scalar.dma_start(xt_sb[:, ko + 1, 0:QC], xt_r[:, ko + 1, 0:QC])
        for ko in range(4, KO):
            nc.gpsimd.dma_start(wk_sb[:, ko, :], wk_r[:, ko, :])
        nc.sync.dma_start(id_sb[:], ident)
        nc.sync.dma_start(tri_sb[:, 0, :], tri)
        # exp activation-table warm-up: off the critical path
        nc.scalar.activation(warm_sb[:], id_sb[:, 0:2], EXP)
        for ko in range(KO):
            nc.sync.dma_start(wq_sb[:, ko, :], wq_r[:, ko, :])
            eng = nc.gpsimd if ko % 2 == 0 else nc.scalar
            eng.dma_start(xt_sb[:, ko, QC:S], xt_r[:, ko, QC:S])
        for ko in range(KO):
            nc.gpsimd.dma_start(wv_sb[:, ko, :], wv_r[:, ko, :])
        nc.sync.dma_start(wo_sb[:], wo_r)
        # ones column of V_aug (softmax denominator accumulator)
        nc.gpsimd.memset(v_sb[:, :, :, DK : DK + 1], 1.0)
        # Q is stored twice with the other head's rows zeroed so score
        # matmuls contract the full 128 partitions from base partition 0
        # (operands at base partition 64 fail on hardware).  The dead halves
        # are zeroed once, per s-chunk, paced so attn(0) is not blocked.
        nc.gpsimd.memset(qt_ev[DK:P, :, 0:QC], 0.0)
        nc.gpsimd.memset(qt_od[0:DK, :, 0:QC], 0.0)

        # ---- building blocks ------------------------------------------
        def drain_k(pj, j, sc):
            nc.vector.tensor_copy(kt_sb[:, j, sc * QC : (sc + 1) * QC], pj[:])

        def drain_q(pj, j, sc):
            nc.vector.tensor_copy(
                qt_ev[0:DK, j, sc * QC : (sc + 1) * QC], pj[0:DK, :])
            nc.vector.tensor_copy(
                qt_od[DK:P, j, sc * QC : (sc + 1) * QC], pj[DK:P, :])

        def emit_kq_group(drain, w_sb, j, sc):
            pj = mixps.tile([P, QC], F32, tag="mix", name="pj")
            for ko in range(KO):
                nc.tensor.matmul(
                    pj[:],
                    w_sb[:, ko, j * P : (j + 1) * P],
                    xt_sb[:, ko, sc * QC : (sc + 1) * QC],
                    start=(ko == 0),
                    stop=(ko == KO - 1),
                )
            drain(pj, j, sc)

        def emit_v_group(kc):
            vp = mixps.tile([P, HD], F32, tag="mix", name="vp")
            for ko in range(KO):
                nc.tensor.matmul(
                    vp[:],
                    xt_sb[:, ko, kc * P : (kc + 1) * P],
                    wv_sb[:, ko, :],
                    start=(ko == 0),
                    stop=(ko == KO - 1),
                )
            nc.vector.tensor_copy(
                v_sb[:, kc, :, 0:DK], vp.rearrange("p (h e) -> p h e", h=NH)
            )

        def emit_oproj_half(qs, no):
            op = mixps.tile([P, QC], F32, tag="mix", name="op")
            for j in range(NJ):
                nc.tensor.matmul(
                    op[:],
                    ctxT_sb[:, j, qs, :],
                    wo_sb[:, j, no * QC : (no + 1) * QC],
                    start=(j == 0),
                    stop=(j == NJ - 1),
                )
            o2 = osb.tile([P, QC], BF16, tag="o")
            nc.vector.tensor_copy(o2[:], op[:])
            nc.sync.dma_start(
                out[qs * P : (qs + 1) * P, no * QC : (no + 1) * QC], o2[:]
            )

        def emit_oproj(qs):
            for no in range(2):
                emit_oproj_half(qs, no)

        # ---- prologue: first-chunk projections -------------------------
        for j in range(NJ):
            emit_kq_group(drain_k, wk_sb, j, 0)
        for j in range(NJ):
            emit_kq_group(drain_q, wq_sb, j, 0)
        emit_v_group(0)

        # PE filler emitted inside the attention loops: later chunks'
        # projections plus deferred early output projections, broken into
        # ~0.4us micro-steps (2 matmuls each) so one step per kc exactly
        # covers the PE deficit under the exp-bound (ACT 1.04us/kc) stretches.
        # Each (deadline, thunk) step must run before attn(deadline) starts.
        from collections import deque

        def kq_steps(drain_fn, w_sb, j, sc, dl):
            st = {}

            def mk(k0):
                def f():
                    if k0 == 0:
                        st["pj"] = mixps.tile([P, QC], F32, tag="mix", name="pj")
                    for ko in (k0, k0 + 1):
                        nc.tensor.matmul(
                            st["pj"][:],
                            w_sb[:, ko, j * P : (j + 1) * P],
                            xt_sb[:, ko, sc * QC : (sc + 1) * QC],
                            start=(ko == 0),
                            stop=(ko == KO - 1),
                        )
                return f

            def drain():
                drain_fn(st["pj"], j, sc)

            return [(dl, mk(k)) for k in (0, 2, 4, 6)] + [(dl, drain)]

        def v_steps(kc, dl):
            st = {}

            def mk(k0):
                def f():
                    if k0 == 0:
                        st["vp"] = mixps.tile([P, HD], F32, tag="mix", name="vp")
                    for ko in (k0, k0 + 1):
                        nc.tensor.matmul(
                            st["vp"][:],
                            xt_sb[:, ko, kc * P : (kc + 1) * P],
                            wv_sb[:, ko, :],
                            start=(ko == 0),
                            stop=(ko == KO - 1),
                        )
                return f

            def drain():
                nc.vector.tensor_copy(
                    v_sb[:, kc, :, 0:DK],
                    st["vp"].rearrange("p (h e) -> p h e", h=NH))

            return [(dl, mk(k)) for k in (0, 2, 4, 6)] + [(dl, drain)]

        def op_steps(oqs, no, dl):
            st = {}

            def mk(j0):
                def f():
                    if j0 == 0:
                        st["op"] = mixps.tile([P, QC], F32, tag="mix", name="op")
                    for j in (j0, j0 + 1):
                        nc.tensor.matmul(
                            st["op"][:],
                            ctxT_sb[:, j, oqs, :],
                            wo_sb[:, j, no * QC : (no + 1) * QC],
                            start=(j == 0),
                            stop=(j == NJ - 1),
                        )
                return f

            def drain():
                o2 = osb.tile([P, QC], BF16, tag="o", name="o2")
                nc.vector.tensor_copy(o2[:], st["op"][:])
                nc.sync.dma_start(
                    out[oqs * P : (oqs + 1) * P, no * QC : (no + 1) * QC],
                    o2[:])

            return [(dl, mk(0)), (dl, mk(2)), (dl, drain)]

        def qzero_step(sc, dl):
            def f():
                nc.gpsimd.memset(qt_ev[DK:P, :, sc * QC : (sc + 1) * QC], 0.0)
                nc.gpsimd.memset(qt_od[0:DK, :, sc * QC : (sc + 1) * QC], 0.0)
            return [(dl, f)]

        steps = deque()
        for sc in (1, 2, 3):
            lo = 4 * sc
            steps.extend(qzero_step(sc, lo - 2))
            for kc in range(lo - 3, lo):
                steps.extend(v_steps(kc, kc))
            for j in range(NJ):
                steps.extend(kq_steps(drain_k, wk_sb, j, sc, lo))
            for j in range(NJ):
                steps.extend(kq_steps(drain_q, wq_sb, j, sc, lo))
            steps.extend(v_steps(lo, lo))
        for kc in (13, 14, 15):
            steps.extend(v_steps(kc, kc))
        for oqs in (0, 1, 2, 3, 4, 6, 7, 8, 9):  # OP(5) held as tail filler
            steps.extend(op_steps(oqs, 0, 99))
            steps.extend(op_steps(oqs, 1, 99))

        tri_b = tri_sb.broadcast_to([P, NH, P])

        def emit_norm(qs, cx4):
            # normalize (q is the partition dim -> broadcast along free)
            rr = rrp.tile([P, 2, 4, 1], F32, tag="rr", name="rr")
            nc.vector.reciprocal(rr[:], cx4[:, :, :, DK : DK + 1])
            cn = cnp.tile([P, 2, 4, DK], BF16, tag="cn", name="cn")
            nc.vector.tensor_mul(
                cn[:], cx4[:, :, :, 0:DK], rr.broadcast_to([P, 2, 4, DK]))
            return cn

        def emit_transp(qs, cn):
            # transpose to pair layout: one PE transpose per head pair
            tp = mixps.tile([P, NJ, P], BF16, tag="mix", name="tp")
            for j in range(NJ):
                nc.tensor.matmul(
                    tp[:, j, :],
                    cn[:, (2 * j) // 4, (2 * j) % 4 : (2 * j) % 4 + 2, :],
                    id_sb[:],
                    is_transpose=True,
                    start=True,
                    stop=True,
                    skip_group_check=True,
                )
            nc.vector.tensor_copy(ctxT_sb[:, :, qs, :], tp[:])

        # ---- main q-tile sweep ------------------------------------------
        # The norm/transpose/oproj of q-tile qs is deferred into the first kc
        # steps of qs+1, so PE never waits on the DVE norm chain at a q-tile
        # boundary: the next tile's scores run under it.
        import os
        nqs_lim = int(os.environ.get("NQS_LIM", NQS))
        pending = None  # (qs, cx4) awaiting norm/transpose/output-projection
        for qs in range(nqs_lim):
            nkc = qs + 1
            pcn = None
            if pending is not None:
                pcn = emit_norm(pending[0], pending[1])  # DVE only
            # 4 heads per 512-f32 PSUM bank (65-wide groups must not cross a
            # bank boundary): head h lives at cx4[:, h//4, h%4, :]
            cx = cxps.tile([P, 2, 512], F32, tag="cx", name="cx")
            cx4 = cx[:, :, 0 : 4 * (DK + 1)].rearrange(
                "p b (h e) -> p b h e", h=4)
            nc.vector.memset(cx4[:], 0.0)
            # force any filler whose deadline has arrived
            while steps and steps[0][0] <= qs:
                steps.popleft()[1]()
            for kc in range(nkc):
                sp = spps.tile([P, NH, P], F32, tag="sp", name="sp")
                for h in range(NH):
                    j = h // 2
                    qsrc = qt_ev if h % 2 == 0 else qt_od
                    nc.tensor.matmul(
                        sp[:, h, :],
                        kt_sb[:, j, kc * P : (kc + 1) * P],
                        qsrc[:, j, qs * P : (qs + 1) * P],
                        start=True,
                        stop=True,
                        skip_group_check=True,
                    )
                pt = ptp.tile([P, NH, P], BF16, tag="pt", name="pt")
                nc.scalar.activation(pt[:], sp[:], EXP)
                if kc == qs:
                    nc.vector.tensor_mul(pt[:], pt[:], tri_b)
                if pending is not None:
                    # place the deferred transpose/oproj deep enough into this
                    # tile's kc steps that the DVE norm chain and the ctxT
                    # copy complete under preceding PE work
                    if kc == min(1, nkc - 1):
                        emit_transp(pending[0], pcn)
                    if pending[0] >= 10:
                        if kc == 2:
                            emit_oproj_half(pending[0], 0)
                        if kc == 3:
                            emit_oproj_half(pending[0], 1)
                # filler micro-steps sized to this kc step's PE deficit under
                # the exp rate: extra at the tile boundary (kc 0), none where
                # the inline oproj halves already fill (kc 2-3)
                if kc == 0:
                    want = 2
                elif kc in (2, 3) and pending is not None and pending[0] >= 10:
                    want = 0
                else:
                    want = 1
                for _ in range(want):
                    if steps:
                        steps.popleft()[1]()
                for h in range(NH):
                    nc.tensor.matmul(
                        cx4[:, h // 4, h % 4, :],
                        pt[:, h, :],
                        v_sb[:, kc, h, :],
                        start=False,
                        stop=(kc == nkc - 1),
                        skip_group_check=True,
                    )
            pending = (qs, cx4)
        # tail: last q-tile's norm/transpose/projection, with the held-back
        # OP(5) (plus any queue remainder) giving PE work while the DVE norm
        # chain and ctxT copy land
        if pending is not None and nqs_lim == NQS:
            cn15 = emit_norm(pending[0], pending[1])
            while steps:
                steps.popleft()[1]()
            emit_oproj_half(5, 0)
            emit_transp(pending[0], cn15)
            emit_oproj_half(5, 1)
            emit_oproj(pending[0])


def build_nc():
    nc = bacc.Bacc("TRN2", target_bir_lowering=False, debug=False)
    xt = nc.dram_tensor("xt", [D, S], BF16, kind="ExternalInput").ap()
    wq = nc.dram_tensor("wq", [D, HD], BF16, kind="ExternalInput").ap()
    wk = nc.dram_tensor("wk", [D, HD], BF16, kind="ExternalInput").ap()
    wv = nc.dram_tensor("wv", [D, HD], BF16, kind="ExternalInput").ap()
    wo = nc.dram_tensor("wo", [HD, D], BF16, kind="ExternalInput").ap()
    tri = nc.dram_tensor("tri", [P, P], BF16, kind="ExternalInput").ap()
    ident = nc.dram_tensor("ident", [P, P], BF16, kind="ExternalInput").ap()
    out = nc.dram_tensor("out", [S, D], BF16, kind="ExternalOutput").ap()
    with tile.TileContext(nc) as tc:
        with ExitStack() as ctx:
            with nc.allow_low_precision(reason="bf16 kernel by design"):
                _emit(ctx, tc, xt, wq, wk, wv, wo, tri, ident, out)
    nc.compile()
    return nc


def make_in_maps(x, W_q, W_k, W_v, W_o):
    import ml_dtypes

    BF = ml_dtypes.bfloat16
    x = np.asarray(x, dtype=np.float32)
    # fold the 1/sqrt(dk)=1/8 softmax scale into W_q (exact power of two)
    WqT = np.ascontiguousarray(np.asarray(W_q, np.float32).T * 0.125).astype(BF)
    WkT = np.ascontiguousarray(np.asarray(W_k, np.float32).T).astype(BF)
    WvT = np.ascontiguousarray(np.asarray(W_v, np.float32).T).astype(BF)
    WoT = np.ascontiguousarray(np.asarray(W_o, np.float32).T).astype(BF)
    # tri[k, q] = 1 where q >= k (within a diagonal 128x128 block)
    tri = np.triu(np.ones((P, P), np.float32)).astype(BF)
    ident = np.eye(P, dtype=np.float32).astype(BF)
    in_maps = []
    for c in range(2 * B):
        b, g = c // 2, c % 2
        in_maps.append({
            "xt": np.ascontiguousarray(x[b].T).astype(BF),
            "wq": np.ascontiguousarray(WqT[:, g * HD : (g + 1) * HD]),
            "wk": np.ascontiguousarray(WkT[:, g * HD : (g + 1) * HD]),
            "wv": np.ascontiguousarray(WvT[:, g * HD : (g + 1) * HD]),
            "wo": np.ascontiguousarray(WoT[g * HD : (g + 1) * HD, :]),
            "tri": tri,
            "ident": ident,
        })
    return in_maps


def get_runner():
    """Build (once) and cache a jitted 8-core executor for the bass program.

    Returns run(in_maps) -> list of per-core {name: np.ndarray} outputs.
    Mirrors concourse.bass2jax.run_bass_via_pjrt but caches the jitted
    callable so repeat kernel() calls skip re-lowering/compiling.
    """
    if "runner" in _CACHE:
        return _CACHE["runner"]
    import jax
    from jax.experimental.shard_map import shard_map
    from jax.sharding import Mesh, PartitionSpec
    from concourse import mybir as _mb
    from concourse.bass2jax import (
        _bass_exec_p, install_neuronx_cc_hook, partition_id_tensor)

    install_neuronx_cc_hook()
    nc = build_nc()
    n_cores = 2 * B

    partition_name = (nc.partition_id_tensor.name
                      if nc.partition_id_tensor else None)
    in_names, out_names, out_avals = [], [], []
    for alloc in nc.m.functions[0].allocations:
        if not isinstance(alloc, _mb.MemoryLocationSet):
            continue
        name = alloc.memorylocations[0].name
        if alloc.kind == "ExternalInput":
            if name != partition_name:
                in_names.append(name)
        elif alloc.kind == "ExternalOutput":
            out_names.append(name)
            out_avals.append(jax.core.ShapedArray(
                tuple(alloc.tensor_shape), _mb.dt.np(alloc.dtype)))
    n_params = len(in_names)
    all_names = in_names + out_names
    if partition_name is not None:
        all_names = all_names + [partition_name]

    def _body(*args):
        operands = list(args)
        if partition_name is not None:
            operands.append(partition_id_tensor())
        outs = _bass_exec_p.bind(
            *operands,
            out_avals=tuple(out_avals),
            in_names=tuple(all_names),
            out_names=tuple(out_names),
            lowering_input_output_aliases=(),
            sim_require_finite=True,
            sim_require_nnan=True,
            nc=nc,
        )
        return tuple(outs)

    devices = jax.devices()[:n_cores]
    mesh = Mesh(np.asarray(devices), ("core",))
    n_outs = len(out_names)
    sharded = jax.jit(
        shard_map(
            _body, mesh=mesh,
            in_specs=(PartitionSpec("core"),) * (n_params + n_outs),
            out_specs=(PartitionSpec("core"),) * n_outs,
            check_rep=False,
        ),
        donate_argnums=tuple(range(n_params, n_params + n_outs)),
        keep_unused=True,
    )

    def run(in_maps, device_arrays=None):
        concat_in = device_arrays if device_arrays is not None else [
            np.concatenate([np.asarray(in_maps[c][i_name])
                            for c in range(n_cores)], axis=0)
            for i_name in in_names
        ]
        concat_zeros = [
            np.zeros((n_cores * av.shape[0], *av.shape[1:]), av.dtype)
            for av in out_avals
        ]
        out_arrs = sharded(*concat_in, *concat_zeros)
        return [
            {name: np.asarray(out_arrs[i]).reshape(
                n_cores, *out_avals[i].shape)[c]
             for i, name in enumerate(out_names)}
            for c in range(n_cores)
        ]

    _CACHE["runner"] = (run, in_names, out_avals)
    return _CACHE["runner"]


def _run_cores(in_maps):
    """Execute the 8-core program; returns per-core {name: np.ndarray}."""
    from concourse._compat import axon_active
    if axon_active():
        # remote-accelerator proxy: use the cached jitted PJRT executor so
        # repeat calls skip re-lowering/compiling
        run, _, _ = get_runner()
        return run(in_maps)
    # native path (local /dev/neuron*): run_bass_kernel_spmd handles NEFF
    # compile caching + device execution directly
    if "nc" not in _CACHE:
        _CACHE["nc"] = build_nc()
    res = run_bass_kernel_spmd(_CACHE["nc"], in_maps, core_ids=list(range(2 * B)))
    _CACHE["last_exec_time_ns"] = res.exec_time_ns
    return res.results


def kernel(x, W_q, W_k, W_v, W_o):
    in_maps = make_in_maps(x, W_q, W_k, W_v, W_o)
    results = _run_cores(in_maps)
    out = np.empty((B, S, D), np.float32)
    for b in range(B):
        out[b] = (results[2 * b]["out"].astype(np.float32)
                  + results[2 * b + 1]["out"].astype(np.float32))
    return out



# revision 2
# speedup vs baseline: 1.0272x; 1.0272x over previous
"""Multi-head causal self-attention on 8 Trainium2 NeuronCores.

Problem: B=4, S=2048, D=1024, H=16 heads (dk=64), fp32 in/out, causal softmax.

Sharding: hybrid batch x head-group. Core c handles batch b = c//2 and head
group g = c%2 (8 heads = 512 dims). Each core computes QKV projections for
its head group, causal attention, and a partial output projection over its
512 context dims. The host sums the two bf16 partials per batch in fp32.

Device-side design (per core):
  - QKV projections run in fp8e4m3 DoubleRow mode with hi+lo error
    compensation: x and each W are split host-side into hi = e4m3(t) and
    lo = e4m3(t - hi) (power-of-2 pre-scales keep values in e4m3's normal
    range).  Per 128-slab of the 1024-dim contraction, 1.5 DoubleRow
    matmuls replace 1 bf16 matmul: slots (x_hi, W_hi)+(x_lo, W_hi) give
    x~@W_hi at 0.5 cycles/row, and slabs pair up their (x_hi, W_lo)
    corrections two-per-matmul.  0.75x the PE cost of bf16 at ~bf16
    accuracy (the dropped x_lo@W_lo term is ~0.07%^2).
  - Everything else runs fp16 (same PE cost as bf16, ~4 extra mantissa
    bits): scores Q^T/K^T via the zero-padded head-pair trick, exp on ACT
    with the whole scale chain folded into activation scale=2^-25 and
    bias=-4ln2 (pt = exp(s)/16), P@V transposed with a 2048-valued ones
    column so the reciprocal-normalize lands ctx at scale 1.
  - Schedule: forward q-tile sweep; K/Q/V projection groups and deferred
    output projections interleaved into the attention loops as PE filler
    under the exp-bound (ACT ~1.0us/kc) stretches.
"""

import numpy as np
from contextlib import ExitStack

import concourse.bass as bass
import concourse.tile as tile
from concourse import bacc, mybir
from concourse.bass_utils import run_bass_kernel_spmd

B, S, D = 4, 2048, 1024
H16 = 16
DK = 64
G = 2               # head groups (cores per batch)
HD = D // G         # per-core head dims = 512 (8 heads)
NH = HD // DK       # heads per core = 8
NJ = NH // 2        # head pairs per core = 4
P = 128
NQS = S // P        # 16 q subtiles
NKC = S // P        # 16 k chunks
KO = D // P         # 8 contraction chunks for projections
QC = 512            # projection s-chunk
HC = 256            # DoubleRow rhs half-chunk (keeps moving free <= 512)

F32 = mybir.dt.float32
F16 = mybir.dt.float16
BF16 = mybir.dt.bfloat16
E4 = mybir.dt.float8e4
DR = mybir.MatmulPerfMode.DoubleRow
EXP = mybir.ActivationFunctionType.Exp

XS = 8.0            # host pre-scale on x before e4m3 split
WS = 256.0          # host pre-scale on W_q/W_k/W_v before e4m3 split
# scores psum = (XS*WS)^2 * (q.k); fold that and the softmax /8 into ACT
ACT_SCALE = 1.0 / ((XS * WS) ** 2 * 8.0)     # exact power of two: 2^-25
ACT_BIAS = -2.772588722239781                # -4 ln2: pt = exp(s)/16
ONES_VAL = 2048.0   # ones column: den = 2048*sum(pt) cancels v_sb's 2048

_CACHE: dict = {}


def _emit(ctx: ExitStack, tc, x8, wqh, wql, wkh, wkl, wvh, wvl, wo, tri,
          ident, out):
    nc = tc.nc

    persist = ctx.enter_context(tc.tile_pool(name="persist", bufs=1))
    x8_sb = persist.tile([P, KO, 2, S], E4)
    wqh_sb = persist.tile([P, KO, HD], E4)
    wql_sb = persist.tile([P, KO, HD], E4)
    wkh_sb = persist.tile([P, KO, HD], E4)
    wkl_sb = persist.tile([P, KO, HD], E4)
    wvh_sb = persist.tile([P, KO, 2, HD], E4)   # hi duplicated host-side
    wvl_sb = persist.tile([P, KO, HD], E4)
    wo_sb = persist.tile([P, NJ, D], F16)
    qt_ev = persist.tile([P, NJ, S], F16)
    qt_od = persist.tile([P, NJ, S], F16)
    kt_sb = persist.tile([P, NJ, S], F16)
    v_sb = persist.tile([P, NKC, NH, DK + 1], F16)
    ctxT_sb = persist.tile([P, NJ, NQS, P], F16)
    tri_sb = persist.tile([P, 1, P], F16)
    id_sb = persist.tile([P, P], F16)
    bias_sb = persist.tile([P, 1], F32)
    warm_sb = persist.tile([P, 2], F16)

    x8_r = x8.rearrange("(o p) t s -> p o t s", p=P)
    wqh_r = wqh.rearrange("(o p) m -> p o m", p=P)
    wql_r = wql.rearrange("(o p) m -> p o m", p=P)
    wkh_r = wkh.rearrange("(o p) m -> p o m", p=P)
    wkl_r = wkl.rearrange("(o p) m -> p o m", p=P)
    wvh_r = wvh.rearrange("(o p) t m -> p o t m", p=P)
    wvl_r = wvl.rearrange("(o p) m -> p o m", p=P)
    wo_r = wo.rearrange("(j p) o -> p j o", p=P)

    with (
        tc.tile_pool(name="spps", bufs=2, space="PSUM") as spps,
        tc.tile_pool(name="cxps", bufs=1, space="PSUM") as cxps,
        tc.tile_pool(name="mixps", bufs=2, space="PSUM") as mixps,
        tc.tile_pool(name="ptp", bufs=3) as ptp,
        tc.tile_pool(name="cnp", bufs=2) as cnp,
        tc.tile_pool(name="rrp", bufs=2) as rrp,
        tc.tile_pool(name="osb", bufs=3) as osb,
    ):
        # ---- input DMAs, interleaved so the first K-proj group is fully fed
        # early.  The prologue needs wk (hi+lo) + the first s-chunk of x, so
        # those columns load first, the remainder streams in behind.
        for ko in range(4):
            nc.sync.dma_start(wkh_sb[:, ko, :], wkh_r[:, ko, :])
            nc.sync.dma_start(wkl_sb[:, ko, :], wkl_r[:, ko, :])
        for ko in range(0, KO, 2):
            nc.gpsimd.dma_start(x8_sb[:, ko, :, 0:QC], x8_r[:, ko, :, 0:QC])
            nc.scalar.dma_start(
                x8_sb[:, ko + 1, :, 0:QC], x8_r[:, ko + 1, :, 0:QC])
        for ko in range(4, KO):
            nc.gpsimd.dma_start(wkh_sb[:, ko, :], wkh_r[:, ko, :])
            nc.gpsimd.dma_start(wkl_sb[:, ko, :], wkl_r[:, ko, :])
        nc.sync.dma_start(id_sb[:], ident)
        nc.sync.dma_start(tri_sb[:, 0, :], tri)
        nc.gpsimd.memset(bias_sb[:], ACT_BIAS)
        # exp activation-table warm-up: off the critical path
        nc.scalar.activation(warm_sb[:], id_sb[:, 0:2], EXP,
                             bias=bias_sb[:], scale=ACT_SCALE)
        for ko in range(KO):
            nc.sync.dma_start(wqh_sb[:, ko, :], wqh_r[:, ko, :])
            nc.sync.dma_start(wql_sb[:, ko, :], wql_r[:, ko, :])
            eng = nc.gpsimd if ko % 2 == 0 else nc.scalar
            eng.dma_start(x8_sb[:, ko, :, QC:S], x8_r[:, ko, :, QC:S])
        for ko in range(KO):
            nc.gpsimd.dma_start(wvh_sb[:, ko, :, :], wvh_r[:, ko, :, :])
            nc.gpsimd.dma_start(wvl_sb[:, ko, :], wvl_r[:, ko, :])
        nc.sync.dma_start(wo_sb[:], wo_r)
        # ones column of V_aug (softmax denominator accumulator)
        nc.gpsimd.memset(v_sb[:, :, :, DK : DK + 1], ONES_VAL)
        # Q is stored twice with the other head's rows zeroed so score
        # matmuls contract the full 128 partitions from base partition 0.
        nc.gpsimd.memset(qt_ev[DK:P, :, 0:QC], 0.0)
        nc.gpsimd.memset(qt_od[0:DK, :, 0:QC], 0.0)

        # ---- building blocks ------------------------------------------
        def drain_k(pj, j, sc):
            nc.vector.tensor_copy(kt_sb[:, j, sc * QC : (sc + 1) * QC], pj[:])

        def drain_q(pj, j, sc):
            nc.vector.tensor_copy(
                qt_ev[0:DK, j, sc * QC : (sc + 1) * QC], pj[0:DK, :])
            nc.vector.tensor_copy(
                qt_od[DK:P, j, sc * QC : (sc + 1) * QC], pj[DK:P, :])

        # DoubleRow hi/lo K/Q projection group: out pj [P, QC] accumulates
        # two independent 256-wide half column groups, 12 DR matmuls each.
        def kq_mm(pj, wh_sb, wl_sb, j, sc, half, i):
            c0 = sc * QC + half * HC
            o0 = half * HC
            if i < KO:   # hi matmul, slab i: slots (x_hi, W_hi)+(x_lo, W_hi)
                nc.tensor.matmul(
                    pj[:, o0 : o0 + HC],
                    wh_sb[:, i, j * P : (j + 1) * P]
                        .rearrange("p (one m) -> p one m", one=1)
                        .broadcast_to([P, 2, P]),
                    x8_sb[:, i, :, c0 : c0 + HC],
                    start=(i == 0), stop=False, perf_mode=DR,
                )
            else:        # correction matmul, slab pair: (x_hi, W_lo) x2
                kp = i - KO
                nc.tensor.matmul(
                    pj[:, o0 : o0 + HC],
                    wl_sb[:, 2 * kp : 2 * kp + 2, j * P : (j + 1) * P],
                    x8_sb[:, 2 * kp : 2 * kp + 2, 0, c0 : c0 + HC],
                    start=False, stop=(kp == KO // 2 - 1), perf_mode=DR,
                )

        def emit_kq_group(drain, wh_sb, wl_sb, j, sc):
            pj = mixps.tile([P, QC], F32, tag="mix", name="pj")
            for half in range(2):
                for i in range(KO + KO // 2):
                    kq_mm(pj, wh_sb, wl_sb, j, sc, half, i)
            drain(pj, j, sc)

        # V projection group: same 12-DR-matmul structure per 256-half.
        def v_mm(vp, kc, half, i):
            o0 = half * HC
            if i < KO:
                nc.tensor.matmul(
                    vp[:, o0 : o0 + HC],
                    x8_sb[:, i, :, kc * P : (kc + 1) * P],
                    wvh_sb[:, i, :, o0 : o0 + HC],
                    start=(i == 0), stop=False, perf_mode=DR,
                )
            else:
                kp = i - KO
                nc.tensor.matmul(
                    vp[:, o0 : o0 + HC],
                    x8_sb[:, 2 * kp : 2 * kp + 2, 0, kc * P : (kc + 1) * P],
                    wvl_sb[:, 2 * kp : 2 * kp + 2, o0 : o0 + HC],
                    start=False, stop=(kp == KO // 2 - 1), perf_mode=DR,
                )

        def emit_v_group(kc):
            vp = mixps.tile([P, HD], F32, tag="mix", name="vp")
            for half in range(2):
                for i in range(KO + KO // 2):
                    v_mm(vp, kc, half, i)
            nc.vector.tensor_copy(
                v_sb[:, kc, :, 0:DK], vp.rearrange("p (h e) -> p h e", h=NH)
            )

        def emit_oproj_half(qs, no):
            op = mixps.tile([P, QC], F32, tag="mix", name="op")
            for j in range(NJ):
                nc.tensor.matmul(
                    op[:],
                    ctxT_sb[:, j, qs, :],
                    wo_sb[:, j, no * QC : (no + 1) * QC],
                    start=(j == 0),
                    stop=(j == NJ - 1),
                )
            o2 = osb.tile([P, QC], BF16, tag="o")
            nc.vector.tensor_copy(o2[:], op[:])
            nc.sync.dma_start(
                out[qs * P : (qs + 1) * P, no * QC : (no + 1) * QC], o2[:]
            )

        def emit_oproj(qs):
            for no in range(2):
                emit_oproj_half(qs, no)

        # ---- prologue: first-chunk projections -------------------------
        for j in range(NJ):
            emit_kq_group(drain_k, wkh_sb, wkl_sb, j, 0)
        for j in range(NJ):
            emit_kq_group(drain_q, wqh_sb, wql_sb, j, 0)
        emit_v_group(0)

        # PE filler emitted inside the attention loops: later chunks'
        # projections plus deferred early output projections, broken into
        # ~0.3us micro-steps (6 DR matmuls each) so steps per kc cover the
        # PE deficit under the exp-bound (ACT ~1.0us/kc) stretches.
        from collections import deque

        def kq_steps(drain_fn, wh_sb, wl_sb, j, sc, dl):
            st = {}

            def mk(step):
                def f():
                    if step == 0:
                        st["pj"] = mixps.tile([P, QC], F32, tag="mix", name="pj")
                    half = step // 2
                    for i in range(6 * (step % 2), 6 * (step % 2) + 6):
                        kq_mm(st["pj"], wh_sb, wl_sb, j, sc, half, i)
                return f

            def drain():
                drain_fn(st["pj"], j, sc)

            return [(dl, mk(k)) for k in range(4)] + [(dl, drain)]

        def v_steps(kc, dl):
            st = {}

            def mk(step):
                def f():
                    if step == 0:
                        st["vp"] = mixps.tile([P, HD], F32, tag="mix", name="vp")
                    half = step // 2
                    for i in range(6 * (step % 2), 6 * (step % 2) + 6):
                        v_mm(st["vp"], kc, half, i)
                return f

            def drain():
                nc.vector.tensor_copy(
                    v_sb[:, kc, :, 0:DK],
                    st["vp"].rearrange("p (h e) -> p h e", h=NH))

            return [(dl, mk(k)) for k in range(4)] + [(dl, drain)]

        def op_steps(oqs, no, dl):
            st = {}

            def mk(j0):
                def f():
                    if j0 == 0:
                        st["op"] = mixps.tile([P, QC], F32, tag="mix", name="op")
                    for j in (j0, j0 + 1):
                        nc.tensor.matmul(
                            st["op"][:],
                            ctxT_sb[:, j, oqs, :],
                            wo_sb[:, j, no * QC : (no + 1) * QC],
                            start=(j == 0),
                            stop=(j == NJ - 1),
                        )
                return f

            def drain():
                o2 = osb.tile([P, QC], BF16, tag="o", name="o2")
                nc.vector.tensor_copy(o2[:], st["op"][:])
                nc.sync.dma_start(
                    out[oqs * P : (oqs + 1) * P, no * QC : (no + 1) * QC],
                    o2[:])

            return [(dl, mk(0)), (dl, mk(2)), (dl, drain)]

        def qzero_step(sc, dl):
            def f():
                nc.gpsimd.memset(qt_ev[DK:P, :, sc * QC : (sc + 1) * QC], 0.0)
                nc.gpsimd.memset(qt_od[0:DK, :, sc * QC : (sc + 1) * QC], 0.0)
            return [(dl, f)]

        steps = deque()
        for sc in (1, 2, 3):
            lo = 4 * sc
            steps.extend(qzero_step(sc, lo - 2))
            for kc in range(lo - 3, lo):
                steps.extend(v_steps(kc, kc))
            for j in range(NJ):
                steps.extend(kq_steps(drain_k, wkh_sb, wkl_sb, j, sc, lo))
            for j in range(NJ):
                steps.extend(kq_steps(drain_q, wqh_sb, wql_sb, j, sc, lo))
            steps.extend(v_steps(lo, lo))
        for kc in (13, 14, 15):
            steps.extend(v_steps(kc, kc))
        for oqs in (0, 1, 2, 3, 4, 6, 7, 8, 9):  # OP(5) held as tail filler
            steps.extend(op_steps(oqs, 0, 99))
            steps.extend(op_steps(oqs, 1, 99))

        tri_b = tri_sb.broadcast_to([P, NH, P])

        def emit_norm(qs, cx4):
            # normalize (q is the partition dim -> broadcast along free)
            rr = rrp.tile([P, 2, 4, 1], F32, tag="rr", name="rr")
            nc.vector.reciprocal(rr[:], cx4[:, :, :, DK : DK + 1])
            cn = cnp.tile([P, 2, 4, DK], F16, tag="cn", name="cn")
            nc.vector.tensor_mul(
                cn[:], cx4[:, :, :, 0:DK], rr.broadcast_to([P, 2, 4, DK]))
            return cn

        def emit_transp(qs, cn):
            # transpose to pair layout: one PE transpose per head pair
            tp = mixps.tile([P, NJ, P], F16, tag="mix", name="tp")
            for j in range(NJ):
                nc.tensor.matmul(
                    tp[:, j, :],
                    cn[:, (2 * j) // 4, (2 * j) % 4 : (2 * j) % 4 + 2, :],
                    id_sb[:],
                    is_transpose=True,
                    start=True,
                    stop=True,
                    skip_group_check=True,
                )
            nc.vector.tensor_copy(ctxT_sb[:, :, qs, :], tp[:])

        # ---- main q-tile sweep ------------------------------------------
        # The norm/transpose/oproj of q-tile qs is deferred into the first kc
        # steps of qs+1, so PE never waits on the DVE norm chain at a q-tile
        # boundary: the next tile's scores run under it.
        import os
        nqs_lim = int(os.environ.get("NQS_LIM", NQS))
        pending = None  # (qs, cx4) awaiting norm/transpose/output-projection
        for qs in range(nqs_lim):
            nkc = qs + 1
            pcn = None
            if pending is not None:
                pcn = emit_norm(pending[0], pending[1])  # DVE only
            # 4 heads per 512-f32 PSUM bank (65-wide groups must not cross a
            # bank boundary): head h lives at cx4[:, h//4, h%4, :]
            cx = cxps.tile([P, 2, 512], F32, tag="cx", name="cx")
            cx4 = cx[:, :, 0 : 4 * (DK + 1)].rearrange(
                "p b (h e) -> p b h e", h=4)
            nc.vector.memset(cx4[:], 0.0)
            # force any filler whose deadline has arrived
            while steps and steps[0][0] <= qs:
                steps.popleft()[1]()
            for kc in range(nkc):
                sp = spps.tile([P, NH, P], F32, tag="sp", name="sp")
                for h in range(NH):
                    j = h // 2
                    qsrc = qt_ev if h % 2 == 0 else qt_od
                    nc.tensor.matmul(
                        sp[:, h, :],
                        kt_sb[:, j, kc * P : (kc + 1) * P],
                        qsrc[:, j, qs * P : (qs + 1) * P],
                        start=True,
                        stop=True,
                        skip_group_check=True,
                    )
                pt = ptp.tile([P, NH, P], F16, tag="pt", name="pt")
                nc.scalar.activation(pt[:], sp[:], EXP,
                                     bias=bias_sb[:], scale=ACT_SCALE)
                if kc == qs:
                    nc.vector.tensor_mul(pt[:], pt[:], tri_b)
                if pending is not None:
                    # place the deferred transpose/oproj deep enough into this
                    # tile's kc steps that the DVE norm chain and the ctxT
                    # copy complete under preceding PE work
                    if kc == min(1, nkc - 1):
                        emit_transp(pending[0], pcn)
                    if pending[0] >= 10:
                        if kc == 2:
                            emit_oproj_half(pending[0], 0)
                        if kc == 3:
                            emit_oproj_half(pending[0], 1)
                # filler micro-steps sized to this kc step's PE deficit under
                # the exp rate: extra at the tile boundary (kc 0), none where
                # the inline oproj halves already fill (kc 2-3)
                if kc == 0:
                    want = 2
                elif kc in (2, 3) and pending is not None and pending[0] >= 10:
                    want = 0
                else:
                    want = 1
                for _ in range(want):
                    if steps:
                        steps.popleft()[1]()
                for h in range(NH):
                    nc.tensor.matmul(
                        cx4[:, h // 4, h % 4, :],
                        pt[:, h, :],
                        v_sb[:, kc, h, :],
                        start=False,
                        stop=(kc == nkc - 1),
                        skip_group_check=True,
                    )
            pending = (qs, cx4)
        # tail: last q-tile's norm/transpose/projection, with the held-back
        # OP(5) (plus any queue remainder) giving PE work while the DVE norm
        # chain and ctxT copy land
        if pending is not None and nqs_lim == NQS:
            cn15 = emit_norm(pending[0], pending[1])
            while steps:
                steps.popleft()[1]()
            emit_oproj_half(5, 0)
            emit_transp(pending[0], cn15)
            emit_oproj_half(5, 1)
            emit_oproj(pending[0])


def build_nc():
    nc = bacc.Bacc("TRN2", target_bir_lowering=False, debug=False)
    x8 = nc.dram_tensor("x8", [D, 2, S], E4, kind="ExternalInput").ap()
    wqh = nc.dram_tensor("wqh", [D, HD], E4, kind="ExternalInput").ap()
    wql = nc.dram_tensor("wql", [D, HD], E4, kind="ExternalInput").ap()
    wkh = nc.dram_tensor("wkh", [D, HD], E4, kind="ExternalInput").ap()
    wkl = nc.dram_tensor("wkl", [D, HD], E4, kind="ExternalInput").ap()
    wvh = nc.dram_tensor("wvh", [D, 2, HD], E4, kind="ExternalInput").ap()
    wvl = nc.dram_tensor("wvl", [D, HD], E4, kind="ExternalInput").ap()
    wo = nc.dram_tensor("wo", [HD, D], F16, kind="ExternalInput").ap()
    tri = nc.dram_tensor("tri", [P, P], F16, kind="ExternalInput").ap()
    ident = nc.dram_tensor("ident", [P, P], F16, kind="ExternalInput").ap()
    out = nc.dram_tensor("out", [S, D], BF16, kind="ExternalOutput").ap()
    with tile.TileContext(nc) as tc:
        with ExitStack() as ctx:
            with nc.allow_low_precision(reason="fp16/fp8 kernel by design"):
                _emit(ctx, tc, x8, wqh, wql, wkh, wkl, wvh, wvl, wo, tri,
                      ident, out)
    nc.compile()
    return nc


def _split_e4(t, scale):
    """hi/lo e4m3 split of t*scale (host-side, round-to-nearest)."""
    import ml_dtypes
    E4n = ml_dtypes.float8_e4m3
    tf = np.asarray(t, np.float32) * scale
    hi = tf.astype(E4n)
    lo = (tf - hi.astype(np.float32)).astype(E4n)
    return hi, lo


def make_in_maps(x, W_q, W_k, W_v, W_o):
    import ml_dtypes

    x = np.asarray(x, dtype=np.float32)
    WqT = np.ascontiguousarray(np.asarray(W_q, np.float32).T)
    WkT = np.ascontiguousarray(np.asarray(W_k, np.float32).T)
    WvT = np.ascontiguousarray(np.asarray(W_v, np.float32).T)
    WoT = np.ascontiguousarray(np.asarray(W_o, np.float32).T).astype(
        np.float16)
    # tri[k, q] = 1 where q >= k (within a diagonal 128x128 block)
    tri = np.triu(np.ones((P, P), np.float32)).astype(np.float16)
    ident = np.eye(P, dtype=np.float32).astype(np.float16)
    in_maps = []
    for c in range(2 * B):
        b, g = c // 2, c % 2
        xh, xl = _split_e4(x[b].T, XS)               # [D, S]
        x8 = np.ascontiguousarray(np.stack([xh, xl], axis=1))  # [D, 2, S]
        wqh, wql = _split_e4(WqT[:, g * HD : (g + 1) * HD], WS)
        wkh, wkl = _split_e4(WkT[:, g * HD : (g + 1) * HD], WS)
        wvh, wvl = _split_e4(WvT[:, g * HD : (g + 1) * HD], WS)
        wvh2 = np.ascontiguousarray(
            np.stack([wvh, wvh], axis=1))            # [D, 2, HD]
        in_maps.append({
            "x8": x8,
            "wqh": np.ascontiguousarray(wqh),
            "wql": np.ascontiguousarray(wql),
            "wkh": np.ascontiguousarray(wkh),
            "wkl": np.ascontiguousarray(wkl),
            "wvh": wvh2,
            "wvl": np.ascontiguousarray(wvl),
            "wo": np.ascontiguousarray(WoT[g * HD : (g + 1) * HD, :]),
            "tri": tri,
            "ident": ident,
        })
    return in_maps


def get_runner():
    """Build (once) and cache a jitted 8-core executor for the bass program.

    Returns run(in_maps) -> list of per-core {name: np.ndarray} outputs.
    Mirrors concourse.bass2jax.run_bass_via_pjrt but caches the jitted
    callable so repeat kernel() calls skip re-lowering/compiling.
    """
    if "runner" in _CACHE:
        return _CACHE["runner"]
    import jax
    from jax.experimental.shard_map import shard_map
    from jax.sharding import Mesh, PartitionSpec
    from concourse import mybir as _mb
    from concourse.bass2jax import (
        _bass_exec_p, install_neuronx_cc_hook, partition_id_tensor)

    install_neuronx_cc_hook()
    nc = build_nc()
    n_cores = 2 * B

    partition_name = (nc.partition_id_tensor.name
                      if nc.partition_id_tensor else None)
    in_names, out_names, out_avals = [], [], []
    for alloc in nc.m.functions[0].allocations:
        if not isinstance(alloc, _mb.MemoryLocationSet):
            continue
        name = alloc.memorylocations[0].name
        if alloc.kind == "ExternalInput":
            if name != partition_name:
                in_names.append(name)
        elif alloc.kind == "ExternalOutput":
            out_names.append(name)
            out_avals.append(jax.core.ShapedArray(
                tuple(alloc.tensor_shape), _mb.dt.np(alloc.dtype)))
    n_params = len(in_names)
    all_names = in_names + out_names
    if partition_name is not None:
        all_names = all_names + [partition_name]

    def _body(*args):
        operands = list(args)
        if partition_name is not None:
            operands.append(partition_id_tensor())
        outs = _bass_exec_p.bind(
            *operands,
            out_avals=tuple(out_avals),
            in_names=tuple(all_names),
            out_names=tuple(out_names),
            lowering_input_output_aliases=(),
            sim_require_finite=True,
            sim_require_nnan=True,
            nc=nc,
        )
        return tuple(outs)

    devices = jax.devices()[:n_cores]
    mesh = Mesh(np.asarray(devices), ("core",))
    n_outs = len(out_names)
    sharded = jax.jit(
        shard_map(
            _body, mesh=mesh,
            in_specs=(PartitionSpec("core"),) * (n_params + n_outs),
            out_specs=(PartitionSpec("core"),) * n_outs,
            check_rep=False,
        ),
        donate_argnums=tuple(range(n_params, n_params + n_outs)),
        keep_unused=True,
    )

    def run(in_maps, device_arrays=None):
        concat_in = device_arrays if device_arrays is not None else [
            np.concatenate([np.asarray(in_maps[c][i_name])
                            for c in range(n_cores)], axis=0)
            for i_name in in_names
        ]
        concat_zeros = [
            np.zeros((n_cores * av.shape[0], *av.shape[1:]), av.dtype)
            for av in out_avals
        ]
        out_arrs = sharded(*concat_in, *concat_zeros)
        return [
            {name: np.asarray(out_arrs[i]).reshape(
                n_cores, *out_avals[i].shape)[c]
             for i, name in enumerate(out_names)}
            for c in range(n_cores)
        ]

    _CACHE["runner"] = (run, in_names, out_avals)
    return _CACHE["runner"]


def _run_cores(in_maps):
    """Execute the 8-core program; returns per-core {name: np.ndarray}."""
    from concourse._compat import axon_active
    if axon_active():
        # remote-accelerator proxy: use the cached jitted PJRT executor so
        # repeat calls skip re-lowering/compiling
        run, _, _ = get_runner()
        return run(in_maps)
    # native path (local /dev/neuron*): run_bass_kernel_spmd handles NEFF
    # compile caching + device execution directly
    if "nc" not in _CACHE:
        _CACHE["nc"] = build_nc()
    res = run_bass_kernel_spmd(_CACHE["nc"], in_maps, core_ids=list(range(2 * B)))
    _CACHE["last_exec_time_ns"] = res.exec_time_ns
    return res.results


def kernel(x, W_q, W_k, W_v, W_o):
    in_maps = make_in_maps(x, W_q, W_k, W_v, W_o)
    results = _run_cores(in_maps)
    out = np.empty((B, S, D), np.float32)
    for b in range(B):
        out[b] = (results[2 * b]["out"].astype(np.float32)
                  + results[2 * b + 1]["out"].astype(np.float32))
    return out
